# revision 30
# baseline (speedup 1.0000x reference)
"""Trainium2 Bass kernel for nn_BertEncoder_57432302682802 (ragged_sequence).

Reference computation (per example):
    scores = hidden @ w_attn + b            # [S]  (b cancels in softmax)
    member[e, s] = (starts[e] <= s <= ends[e]) & valid[e]
    attn = softmax over s of (scores masked to member) * member
    edu[e, :] = sum_s attn[e, s] * hidden[s, :]
Outputs: (hidden passthrough, edu [B, E, H], mask_edu = valid[:, None, :])

Sharding: data-parallel over batch, 8 examples per NeuronCore x 8 cores.
hidden/mask_edu never touch the device (passthrough / host reshape).

Device-side design (per core, 8 examples, all math exact fp32):
  - hidden loads in natural [token, h] layout, 4 chunks of 128 tokens per
    example, spread over three DMA generation paths (Sync/Scalar HWDGE +
    GpSimd SWDGE) so transfers pipeline with compute.
  - scores: one fused DVE scalar_tensor_tensor (multiply + accumulate
    along the free dim) per chunk against a host-broadcast w tile.
    (tensor_tensor_reduce hard-faults this runtime path; scalar_tensor_
    tensor with accum_out is the working equivalent.)
  - exp on ScalarE; attnT[s, e] = memberT[s, e] * exp(scores[s]) also on
    ScalarE (activation Copy, per-partition scale), memberT precomputed
    on the host as uint8 from the tiny start/end/valid arrays.
  - pooling: PE matmuls with lhsT = attnT chunk [128, 32]; four examples
    run in the four 32-wide PE column strips (tile_position=(0, 32j)),
    each accumulating into its own PSUM bank (the per-bank has_written
    clear of start=True stays strip-local).  A ones column appended to
    the hidden tile makes the softmax denominator fall out of the same
    accumulation.  fp32 matmuls cost 4 cyc/col (2 half-speed passes) --
    measured float32r is only ~1e-4 accurate, so exact fp32 is used.
  - a bf16 warmup matmul burst holds the PE HAM un-throttled (2.4 GHz)
    until the first real matmuls arrive.
  - normalization fused into the PSUM->SBUF evacuation: ScalarE Copy
    with scale = 1/(d + eps) for the first 512 columns, DVE tensor_scalar
    for the rest; eps keeps empty/invalid spans at exactly edu == 0.
"""

import os

import numpy as np

import concourse.bacc as bacc
import concourse.bass as bass
import concourse.mybir as mybir
import concourse.tile as tile
from concourse.bass_utils import run_bass_kernel_spmd

B, S, H, E = 64, 512, 768, 32
N_CORES = 8
PER = B // N_CORES           # 8 examples per core
P = 128                      # partitions
NCH = S // P                 # 4 token chunks per example
NSPLIT = 512                 # PSUM bank-sized slice of H
DT = mybir.dt.float32
EPS = 1e-38                  # keeps empty spans at edu == 0 instead of NaN

TRACE = bool(int(os.environ.get("KERNEL_TRACE", "0")))
LAST_RESULTS = None          # test harness reads exec_time_ns from here


def _ensure_ntff_hook():
    """Provide antenv.axon_hooks if the image lacks it (profiling only)."""
    try:
        from antenv.axon_hooks import get_axon_ntff_profile_hook  # noqa: F401

        return
    except ImportError:
        pass
    try:
        import sys
        import types

        import antenv
        from trn_agent_boot.trn_boot import _ntff_profile_via_ctypes

        hook = _ntff_profile_via_ctypes("/opt/axon/libaxon_pjrt.so")
        mod = types.ModuleType("antenv.axon_hooks")
        mod.get_axon_ntff_profile_hook = lambda: hook
        mod.set_axon_ntff_profile_hook = lambda h: None
        sys.modules["antenv.axon_hooks"] = mod
        antenv.axon_hooks = mod
    except Exception:
        pass


def _build_body(tc, hidden, member, wb, edu):
    nc = tc.nc

    HP = H + 1  # hidden chunk + ones column (denominator rides the matmul)
    NB = HP - NSPLIT  # second PSUM split width (257), d in last column
    GSZ = 4  # examples per column-strip group (PE col_grp 32-strips)
    WARMUP = 40

    with (
        tc.tile_pool(name="hid", bufs=PER) as hid_pool,
        tc.tile_pool(name="const", bufs=1) as const_pool,
        tc.tile_pool(name="scratch", bufs=2) as scratch_pool,
        tc.tile_pool(name="small", bufs=4) as small_pool,
        tc.tile_pool(name="attn", bufs=PER) as attn_pool,
        tc.tile_pool(name="edu_sb", bufs=2) as edu_pool,
        tc.tile_pool(name="psA", bufs=4, space="PSUM") as psA_pool,
        tc.tile_pool(name="psB", bufs=4, space="PSUM") as psB_pool,
    ):
        # w broadcast [128, H] first: the scores op needs it immediately.
        wt = const_pool.tile([P, H], DT, name="wt")
        nc.sync.dma_start(wt[:, :], wb)

        # PE warmup: dense bf16 matmuls bridge the HAM throttle window so
        # the real f32 matmuls (which only start once the first group's
        # scores are ready) run at 2.4 GHz from the start.
        wl = const_pool.tile([P, 2], mybir.dt.bfloat16, name="wl")
        wr = const_pool.tile([P, NSPLIT], mybir.dt.bfloat16, name="wr")
        nc.gpsimd.memset(wl[:, :], 0.0)
        nc.gpsimd.memset(wr[:, :], 0.0)
        warm_ps = psA_pool.tile([2, NSPLIT], DT, name="psA")
        for _ in range(WARMUP):
            nc.tensor.matmul(
                warm_ps[:, :], wl[:, :], wr[:, :], start=True, stop=True
            )

        mem = const_pool.tile([P, PER, NCH, E], mybir.dt.uint8, name="mem")
        hids = {}
        attns = {}

        def load_example(ex, eng):
            hid = hid_pool.tile([P, NCH, HP], DT, name="hid")
            src_ap = hidden[ex].rearrange("(c p) h -> p c h", p=P)
            half = NCH // 2
            eng.dma_start(hid[:, 0:half, 0:H], src_ap[:, 0:half, :])
            eng.dma_start(hid[:, half:NCH, 0:H], src_ap[:, half:NCH, :])
            nc.vector.memset(hid[:, :, H : H + 1], 1.0)
            hids[ex] = hid

        def exp_attn(ex, scoresT):
            expT = small_pool.tile([P, NCH], DT, name="expT")
            nc.scalar.activation(
                expT[:, :], scoresT[:, :], mybir.ActivationFunctionType.Exp
            )
            attn = attn_pool.tile([P, NCH, E], DT, name="attn")
            for c in range(NCH):
                nc.scalar.activation(
                    attn[:, c, :], mem[:, ex, c, :],
                    mybir.ActivationFunctionType.Copy,
                    scale=expT[:, c : c + 1],
                )
            attns[ex] = attn

        def scores_attn(ex):
            # scores on the Vector engine (fused multiply+reduce)
            hid = hids[ex]
            scoresT = small_pool.tile([P, NCH], DT, name="scoresT")
            for c in range(NCH):
                scratch = scratch_pool.tile([P, H], DT, name="scratch")
                nc.vector.scalar_tensor_tensor(
                    out=scratch[:, :],
                    in0=hid[:, c, 0:H],
                    scalar=0.0,
                    in1=wt[:, :],
                    op0=mybir.AluOpType.bypass,
                    op1=mybir.AluOpType.mult,
                    accum_out=scoresT[:, c : c + 1],
                )
            exp_attn(ex, scoresT)

        def scores_attn_gp(ex):
            # scores via GpSimd multiply + ScalarE fused accumulate --
            # keeps the last examples off the busy Vector engine entirely
            hid = hids[ex]
            scoresT = small_pool.tile([P, NCH], DT, name="scoresT")
            for c in range(NCH):
                scratch = scratch_pool.tile([P, H], DT, name="gscr")
                nc.gpsimd.tensor_mul(scratch[:, :], hid[:, c, 0:H], wt[:, :])
                scratch2 = scratch_pool.tile([P, H], DT, name="ascr")
                nc.scalar.activation(
                    scratch2[:, :], scratch[:, :],
                    mybir.ActivationFunctionType.Copy,
                    accum_out=scoresT[:, c : c + 1],
                )
            exp_attn(ex, scoresT)

        # DMA plan: Sync ring uncontended early (wt, ex0, member, ex1,
        # ex2); GpSimd SWDGE ring carries ex3 in parallel; Scalar ring
        # triggers ex4-7 once its stream clears the first ACT work --
        # each lands just before its scores op needs it.
        load_example(0, nc.sync)
        nc.sync.dma_start(mem[:, :, :, :], member)
        scores_attn(0)
        load_example(3, nc.gpsimd)
        load_example(1, nc.sync)
        scores_attn(1)
        load_example(2, nc.sync)
        scores_attn(2)
        load_example(4, nc.scalar)
        scores_attn(3)
        load_example(5, nc.scalar)
        scores_attn(4)
        load_example(6, nc.scalar)
        scores_attn(5)
        load_example(7, nc.scalar)
        scores_attn(6)
        scores_attn(7)

        edu_tiles = {}
        for g in range(PER // GSZ):
            exs = list(range(g * GSZ, (g + 1) * GSZ))
            # GSZ examples run concurrently in 32-wide PE column strips.
            # Each strip accumulates in its OWN psum bank so the per-bank
            # has_written clear of start=True never touches another strip.
            pair, half = divmod(g, 2) if GSZ == 2 else (g, 0)
            psAs = [psA_pool.tile([P, NSPLIT], DT, name="psA") for _ in exs]
            psBs = [psB_pool.tile([P, NB], DT, name="psB") for _ in exs]
            # odd groups use PE column strips 2,3 so PSUM partitions line up
            # with their half of the shared output tile (no cross-partition
            # copies; engine lanes are fixed).
            strips = [half * GSZ + j for j in range(GSZ)]
            for c in range(NCH):
                first, last = c == 0, c == NCH - 1
                for j, ex in enumerate(exs):
                    sl = slice(32 * strips[j], 32 * strips[j] + 32)
                    nc.tensor.matmul(
                        psBs[j][sl, :], attns[ex][:, c, :],
                        hids[ex][:, c, NSPLIT:HP],
                        start=first, stop=last,
                        tile_position=(0, 32 * strips[j]),
                    )
                for j, ex in enumerate(exs):
                    sl = slice(32 * strips[j], 32 * strips[j] + 32)
                    nc.tensor.matmul(
                        psAs[j][sl, :], attns[ex][:, c, :],
                        hids[ex][:, c, 0:NSPLIT],
                        start=first, stop=last,
                        tile_position=(0, 32 * strips[j]),
                    )

            # two consecutive groups share one [128, H] output tile so the
            # store DMA runs at full partition width
            if half == 0:
                edu_tiles[pair] = edu_pool.tile([P, H], DT, name="edu_sb")
            edu_sb = edu_tiles[pair]
            dsb = small_pool.tile([P, 1], DT, name="dsb")
            rsb = small_pool.tile([P, 1], DT, name="rsb")
            for j, ex in enumerate(exs):
                sl = slice(32 * strips[j], 32 * strips[j] + 32)
                nc.vector.tensor_scalar_add(
                    dsb[sl, :], psBs[j][sl, NB - 1 : NB], EPS
                )
                nc.vector.reciprocal(rsb[sl, :], dsb[sl, :])
                nc.scalar.activation(
                    edu_sb[sl, 0:NSPLIT], psAs[j][sl, :],
                    mybir.ActivationFunctionType.Copy, scale=rsb[sl, 0:1],
                )
                nc.vector.tensor_scalar_mul(
                    edu_sb[sl, NSPLIT:H], psBs[j][sl, 0 : NB - 1], rsb[sl, 0:1]
                )

            if GSZ == 4 or half == 1:
                lo = pair * (P // E)  # first example index in this tile
                nc.sync.dma_start(
                    edu[lo : lo + P // E].rearrange("x e h -> (x e) h"),
                    edu_sb[:, :],
                )


def build_nc():
    nc = bacc.Bacc(
        "TRN2", target_bir_lowering=False, debug=False, num_devices=N_CORES
    )
    hidden = nc.dram_tensor(
        "hidden", [PER, S, H], DT, kind="ExternalInput"
    ).ap()
    member = nc.dram_tensor(
        "member", [P, PER, NCH, E], mybir.dt.uint8, kind="ExternalInput"
    ).ap()
    wb = nc.dram_tensor("wb", [P, H], DT, kind="ExternalInput").ap()
    edu = nc.dram_tensor("edu", [PER, E, H], DT, kind="ExternalOutput").ap()
    with tile.TileContext(nc) as tc:
        _build_body(tc, hidden, member, wb, edu)
    nc.compile()
    return nc


_NC_CACHE = None


def _get_nc():
    global _NC_CACHE
    if _NC_CACHE is None:
        _NC_CACHE = build_nc()
    return _NC_CACHE


def kernel(hidden, w_attn, b_attn, edu_starts, edu_ends, edu_valid):
    global LAST_RESULTS
    hidden = np.asarray(hidden, dtype=np.float32)
    w_attn = np.asarray(w_attn, dtype=np.float32)
    b_attn = np.asarray(b_attn, dtype=np.float32)
    edu_starts = np.asarray(edu_starts, dtype=np.int32)
    edu_ends = np.asarray(edu_ends, dtype=np.int32)
    edu_valid = np.asarray(edu_valid, dtype=bool)

    # Host prep: membership mask (b_attn cancels inside each span's softmax).
    starts = np.where(edu_valid, edu_starts, S).astype(np.int64)  # [B, E]
    ends = np.where(edu_valid, edu_ends, -1).astype(np.int64)
    pos = np.arange(S, dtype=np.int64)
    member = (
        (pos[None, :, None] >= starts[:, None, :])
        & (pos[None, :, None] <= ends[:, None, :])
    ).astype(np.uint8)                                       # [B, S, E]
    # device layout [128, per-core ex, chunk, E]
    member_dev = member.reshape(N_CORES, PER, NCH, P, E).transpose(0, 3, 1, 2, 4)
    member_dev = np.ascontiguousarray(member_dev)
    wb = np.ascontiguousarray(np.broadcast_to(w_attn[None, :], (P, H)))

    in_maps = [
        {
            "hidden": np.ascontiguousarray(
                hidden[core * PER : (core + 1) * PER]
            ),
            "member": member_dev[core],
            "wb": wb,
        }
        for core in range(N_CORES)
    ]

    nc = _get_nc()
    if TRACE:
        _ensure_ntff_hook()
    LAST_RESULTS = run_bass_kernel_spmd(
        nc, in_maps, core_ids=list(range(N_CORES)), trace=TRACE
    )
    edu = np.concatenate(
        [r["edu"] for r in LAST_RESULTS.results], axis=0
    ).reshape(B, E, H)

    mask_edu = edu_valid[:, None, :]
    return hidden, edu, mask_edu


if __name__ == "__main__":
    import reference

    inputs = {k: np.asarray(v) for k, v in reference.setup_inputs().items()}
    outs = kernel(**inputs)
    print([(o.shape, o.dtype) for o in outs])


# revision 32
# speedup vs baseline: 1.0692x; 1.0692x over previous
"""Trainium2 Bass kernel for nn_BertEncoder_57432302682802 (ragged_sequence).

Reference computation (per example):
    scores = hidden @ w_attn + b            # [S]  (b cancels in softmax)
    member[e, s] = (starts[e] <= s <= ends[e]) & valid[e]
    attn = softmax over s of (scores masked to member) * member
    edu[e, :] = sum_s attn[e, s] * hidden[s, :]
Outputs: (hidden passthrough, edu [B, E, H], mask_edu = valid[:, None, :])

Sharding: data-parallel over batch, 8 examples per NeuronCore x 8 cores.
hidden/mask_edu never touch the device (passthrough / host reshape).

Device-side design (per core, 8 examples, all math exact fp32):
  - hidden loads in natural [token, h] layout, 4 chunks of 128 tokens per
    example, spread over three DMA generation paths (Sync/Scalar HWDGE +
    GpSimd SWDGE) so transfers pipeline with compute.
  - scores: one fused DVE scalar_tensor_tensor (multiply + accumulate
    along the free dim) per chunk against a host-broadcast w tile.
    (tensor_tensor_reduce hard-faults this runtime path; scalar_tensor_
    tensor with accum_out is the working equivalent.)
  - exp on ScalarE; attnT[s, e] = memberT[s, e] * exp(scores[s]) also on
    ScalarE (activation Copy, per-partition scale), memberT precomputed
    on the host as uint8 from the tiny start/end/valid arrays.
  - pooling: PE matmuls with lhsT = attnT chunk [128, 32]; four examples
    run in the four 32-wide PE column strips (tile_position=(0, 32j)),
    each accumulating into its own PSUM bank (the per-bank has_written
    clear of start=True stays strip-local).  A ones column appended to
    the hidden tile makes the softmax denominator fall out of the same
    accumulation.  fp32 matmuls cost 4 cyc/col (2 half-speed passes) --
    measured float32r is only ~1e-4 accurate, so exact fp32 is used.
  - a bf16 warmup matmul burst holds the PE HAM un-throttled (2.4 GHz)
    until the first real matmuls arrive.
  - normalization fused into the PSUM->SBUF evacuation: ScalarE Copy
    with scale = 1/(d + eps) for the first 512 columns, DVE tensor_scalar
    for the rest; eps keeps empty/invalid spans at exactly edu == 0.
"""

import os

import numpy as np

import concourse.bacc as bacc
import concourse.mybir as mybir
import concourse.tile as tile
from concourse.bass_utils import run_bass_kernel_spmd

B, S, H, E = 64, 512, 768, 32
N_CORES = 8
PER = B // N_CORES           # 8 examples per core
P = 128                      # partitions
NCH = S // P                 # 4 token chunks per example
NSPLIT = 512                 # PSUM bank-sized slice of H
DT = mybir.dt.float32
EPS = 1e-38                  # keeps empty spans at edu == 0 instead of NaN

TRACE = bool(int(os.environ.get("KERNEL_TRACE", "0")))
LAST_RESULTS = None          # test harness reads exec_time_ns from here


def _ensure_ntff_hook():
    """Provide antenv.axon_hooks if the image lacks it (profiling only)."""
    try:
        from antenv.axon_hooks import get_axon_ntff_profile_hook  # noqa: F401

        return
    except ImportError:
        pass
    try:
        import sys
        import types

        import antenv
        from trn_agent_boot.trn_boot import _ntff_profile_via_ctypes

        hook = _ntff_profile_via_ctypes("/opt/axon/libaxon_pjrt.so")
        mod = types.ModuleType("antenv.axon_hooks")
        mod.get_axon_ntff_profile_hook = lambda: hook
        mod.set_axon_ntff_profile_hook = lambda h: None
        sys.modules["antenv.axon_hooks"] = mod
        antenv.axon_hooks = mod
    except Exception:
        pass


def _build_body(tc, hidden, member, wb, edu):
    nc = tc.nc

    HP = H + 1  # hidden chunk + ones column (denominator rides the matmul)
    NB = HP - NSPLIT  # second PSUM split width (257), d in last column
    GSZ = 4  # examples per column-strip group (PE col_grp 32-strips)
    WARMUP = 40

    with (
        tc.tile_pool(name="hid", bufs=PER) as hid_pool,
        tc.tile_pool(name="const", bufs=1) as const_pool,
        tc.tile_pool(name="scratch", bufs=2) as scratch_pool,
        tc.tile_pool(name="small", bufs=4) as small_pool,
        tc.tile_pool(name="attn", bufs=PER) as attn_pool,
        tc.tile_pool(name="edu_sb", bufs=2) as edu_pool,
        tc.tile_pool(name="psA", bufs=4, space="PSUM") as psA_pool,
        tc.tile_pool(name="psB", bufs=4, space="PSUM") as psB_pool,
    ):
        # w broadcast [128, H] first: the scores op needs it immediately.
        wt = const_pool.tile([P, H], DT, name="wt")
        nc.sync.dma_start(wt[:, :], wb)

        # PE warmup: dense bf16 matmuls bridge the HAM throttle window so
        # the real f32 matmuls (which only start once the first group's
        # scores are ready) run at 2.4 GHz from the start.
        wl = const_pool.tile([P, 2], mybir.dt.bfloat16, name="wl")
        wr = const_pool.tile([P, NSPLIT], mybir.dt.bfloat16, name="wr")
        nc.gpsimd.memset(wl[:, :], 0.0)
        nc.gpsimd.memset(wr[:, :], 0.0)
        warm_ps = psA_pool.tile([2, NSPLIT], DT, name="psA")
        for _ in range(WARMUP):
            nc.tensor.matmul(
                warm_ps[:, :], wl[:, :], wr[:, :], start=True, stop=True
            )

        mem = const_pool.tile([P, PER, NCH, E], mybir.dt.uint8, name="mem")
        hids = {}
        attns = {}

        def load_example(ex, eng):
            hid = hid_pool.tile([P, NCH, HP], DT, name="hid")
            src_ap = hidden[ex].rearrange("(c p) h -> p c h", p=P)
            half = NCH // 2
            eng.dma_start(hid[:, 0:half, 0:H], src_ap[:, 0:half, :])
            eng.dma_start(hid[:, half:NCH, 0:H], src_ap[:, half:NCH, :])
            nc.vector.memset(hid[:, :, H : H + 1], 1.0)
            hids[ex] = hid

        def exp_attn(ex, scoresT):
            expT = small_pool.tile([P, NCH], DT, name="expT")
            nc.scalar.activation(
                expT[:, :], scoresT[:, :], mybir.ActivationFunctionType.Exp
            )
            attn = attn_pool.tile([P, NCH, E], DT, name="attn")
            for c in range(NCH):
                nc.scalar.activation(
                    attn[:, c, :], mem[:, ex, c, :],
                    mybir.ActivationFunctionType.Copy,
                    scale=expT[:, c : c + 1],
                )
            attns[ex] = attn

        def scores_attn(ex):
            # scores on the Vector engine (fused multiply+reduce)
            hid = hids[ex]
            scoresT = small_pool.tile([P, NCH], DT, name="scoresT")
            for c in range(NCH):
                scratch = scratch_pool.tile([P, H], DT, name="scratch")
                nc.vector.scalar_tensor_tensor(
                    out=scratch[:, :],
                    in0=hid[:, c, 0:H],
                    scalar=0.0,
                    in1=wt[:, :],
                    op0=mybir.AluOpType.bypass,
                    op1=mybir.AluOpType.mult,
                    accum_out=scoresT[:, c : c + 1],
                )
            exp_attn(ex, scoresT)

        def scores_attn_gp(ex):
            # scores via GpSimd multiply + ScalarE fused accumulate --
            # keeps the last examples off the busy Vector engine entirely
            hid = hids[ex]
            scoresT = small_pool.tile([P, NCH], DT, name="scoresT")
            for c in range(NCH):
                scratch = scratch_pool.tile([P, H], DT, name="gscr")
                nc.gpsimd.tensor_mul(scratch[:, :], hid[:, c, 0:H], wt[:, :])
                scratch2 = scratch_pool.tile([P, H], DT, name="ascr")
                nc.scalar.activation(
                    scratch2[:, :], scratch[:, :],
                    mybir.ActivationFunctionType.Copy,
                    accum_out=scoresT[:, c : c + 1],
                )
            exp_attn(ex, scoresT)

        # DMA plan: Sync ring uncontended early (wt, ex0, member, ex1,
        # ex2); GpSimd SWDGE ring carries ex3 in parallel; Scalar ring
        # triggers ex4-7 once its stream clears the first ACT work --
        # each lands just before its scores op needs it.
        hid0 = hid_pool.tile([P, NCH, HP], DT, name="hid")
        src0 = hidden[0].rearrange("(c p) h -> p c h", p=P)
        for c in range(NCH):
            nc.sync.dma_start(hid0[:, c, 0:H], src0[:, c, :])
        nc.vector.memset(hid0[:, :, H : H + 1], 1.0)
        hids[0] = hid0
        nc.sync.dma_start(mem[:, :, :, :], member)
        scores_attn(0)
        load_example(3, nc.gpsimd)
        load_example(1, nc.sync)
        scores_attn(1)
        load_example(2, nc.sync)
        scores_attn(2)
        load_example(4, nc.scalar)
        scores_attn(3)
        load_example(5, nc.scalar)
        scores_attn(4)
        load_example(6, nc.scalar)
        scores_attn(5)
        load_example(7, nc.scalar)
        scores_attn(6)
        scores_attn(7)

        edu_tiles = {}
        group_specs = [
            ([0, 1, 2, 3], 0, "gA", True, slice(0, 4)),
            ([4, 5], 0, "gB", False, slice(4, 6)),
            ([6, 7], 2, "gB", True, slice(4, 8)),
        ]
        for exs, strip_base, ekey, do_dma, out_sl in group_specs:
            gsz = len(exs)
            psAs = [psA_pool.tile([P, NSPLIT], DT, name="psA") for _ in exs]
            psBs = [psB_pool.tile([P, NB], DT, name="psB") for _ in exs]
            strips = [strip_base + j for j in range(gsz)]
            for c in range(NCH):
                first, last = c == 0, c == NCH - 1
                for j, ex in enumerate(exs):
                    sl = slice(32 * strips[j], 32 * strips[j] + 32)
                    nc.tensor.matmul(
                        psBs[j][sl, :], attns[ex][:, c, :],
                        hids[ex][:, c, NSPLIT:HP],
                        start=first, stop=last,
                        tile_position=(0, 32 * strips[j]),
                    )
                for j, ex in enumerate(exs):
                    sl = slice(32 * strips[j], 32 * strips[j] + 32)
                    nc.tensor.matmul(
                        psAs[j][sl, :], attns[ex][:, c, :],
                        hids[ex][:, c, 0:NSPLIT],
                        start=first, stop=last,
                        tile_position=(0, 32 * strips[j]),
                    )

            if ekey not in edu_tiles:
                edu_tiles[ekey] = edu_pool.tile([P, H], DT, name="edu_sb")
            edu_sb = edu_tiles[ekey]
            dsb = small_pool.tile([P, 1], DT, name="dsb")
            rsb = small_pool.tile([P, 1], DT, name="rsb")
            for j, ex in enumerate(exs):
                sl = slice(32 * strips[j], 32 * strips[j] + 32)
                nc.vector.tensor_scalar_add(
                    dsb[sl, :], psBs[j][sl, NB - 1 : NB], EPS
                )
                nc.vector.reciprocal(rsb[sl, :], dsb[sl, :])
                nc.scalar.activation(
                    edu_sb[sl, 0:NSPLIT], psAs[j][sl, :],
                    mybir.ActivationFunctionType.Copy, scale=rsb[sl, 0:1],
                )
                nc.vector.tensor_scalar_mul(
                    edu_sb[sl, NSPLIT:H], psBs[j][sl, 0 : NB - 1], rsb[sl, 0:1]
                )

            if do_dma:
                nc.sync.dma_start(
                    edu[out_sl].rearrange("x e h -> (x e) h"), edu_sb[:, :]
                )


def build_nc():
    nc = bacc.Bacc(
        "TRN2", target_bir_lowering=False, debug=False, num_devices=N_CORES
    )
    hidden = nc.dram_tensor(
        "hidden", [PER, S, H], DT, kind="ExternalInput"
    ).ap()
    member = nc.dram_tensor(
        "member", [P, PER, NCH, E], mybir.dt.uint8, kind="ExternalInput"
    ).ap()
    wb = nc.dram_tensor("wb", [P, H], DT, kind="ExternalInput").ap()
    edu = nc.dram_tensor("edu", [PER, E, H], DT, kind="ExternalOutput").ap()
    with tile.TileContext(nc) as tc:
        _build_body(tc, hidden, member, wb, edu)
    nc.compile()
    return nc


_NC_CACHE = None


def _get_nc():
    global _NC_CACHE
    if _NC_CACHE is None:
        _NC_CACHE = build_nc()
    return _NC_CACHE


def kernel(hidden, w_attn, b_attn, edu_starts, edu_ends, edu_valid):
    global LAST_RESULTS
    hidden = np.asarray(hidden, dtype=np.float32)
    w_attn = np.asarray(w_attn, dtype=np.float32)
    b_attn = np.asarray(b_attn, dtype=np.float32)
    edu_starts = np.asarray(edu_starts, dtype=np.int32)
    edu_ends = np.asarray(edu_ends, dtype=np.int32)
    edu_valid = np.asarray(edu_valid, dtype=bool)

    # Host prep: membership mask (b_attn cancels inside each span's softmax).
    starts = np.where(edu_valid, edu_starts, S).astype(np.int64)  # [B, E]
    ends = np.where(edu_valid, edu_ends, -1).astype(np.int64)
    pos = np.arange(S, dtype=np.int64)
    member = (
        (pos[None, :, None] >= starts[:, None, :])
        & (pos[None, :, None] <= ends[:, None, :])
    ).astype(np.uint8)                                       # [B, S, E]
    # device layout [128, per-core ex, chunk, E]
    member_dev = member.reshape(N_CORES, PER, NCH, P, E).transpose(0, 3, 1, 2, 4)
    member_dev = np.ascontiguousarray(member_dev)
    wb = np.ascontiguousarray(np.broadcast_to(w_attn[None, :], (P, H)))

    in_maps = [
        {
            "hidden": np.ascontiguousarray(
                hidden[core * PER : (core + 1) * PER]
            ),
            "member": member_dev[core],
            "wb": wb,
        }
        for core in range(N_CORES)
    ]

    nc = _get_nc()
    if TRACE:
        _ensure_ntff_hook()
    LAST_RESULTS = run_bass_kernel_spmd(
        nc, in_maps, core_ids=list(range(N_CORES)), trace=TRACE
    )
    edu = np.concatenate(
        [r["edu"] for r in LAST_RESULTS.results], axis=0
    ).reshape(B, E, H)

    mask_edu = edu_valid[:, None, :]
    return hidden, edu, mask_edu


if __name__ == "__main__":
    import reference

    inputs = {k: np.asarray(v) for k, v in reference.setup_inputs().items()}
    outs = kernel(**inputs)
    print([(o.shape, o.dtype) for o in outs])
